# revision 15
# baseline (speedup 1.0000x reference)
"""Builder + host-side sharding for the causal attention head kernel (v4).

B=4, T=2048, C=768, H=64 on 8 NeuronCores, pure data parallel (no
collectives).  Each core owns one batch element and two query quarters
(zigzag pairing for causal load balance): core 2b+0 -> quarters (0, 3),
core 2b+1 -> quarters (1, 2).

Per-core device inputs (host-prearranged, fp16):
  xts   [768, 2048]  x[b].T columns laid out as slots [A | B | F0 | F1]
                     (A = qa keys, B = qb keys; diagonal slots)
  wkv   [128, 6*128] [Wk | Wv] pre-tiled partition-major (p, c, m)
  wq    [128, 6*64]  Wq pre-tiled partition-major
  kind  [16, 2048]   key-block indicator rows: kind[j, s] = 1 iff
                     s in [128j, 128j+128)
  qmask [16, 1536]   additive mask values vs the three query regions
                     (qa | qb | flex): -30000 where key-block j is not
                     allowed for that query column
  qsel  [128, 2]     col 0 = 1.0 if the flex query region copies quarter
                     a's Q, col 1 = 1.0 if quarter b's (complementary)

Device output (fp16):
  out   [65, 1536]   cols 0:512 = quarter a acc, 512:1024 = quarter b,
                     1024:1536 = flex acc (host adds it to quarter b on
                     even cores / quarter a on odd cores).  Rows 0:64 =
                     unnormalized attention numerator^T, row 64 =
                     softmax denominator.  Host divides.

Unit structure (20 score/PV units per core, all real work):
  tri A x qa   (4, trimmed free 512/384/256/128, triangular diagonal)
  tri B x qb   (4, trimmed)
  full A x qb  (4)   even: q0 keys x q3; odd: q1 keys x q2
  full F0 x qb (4)   even: q1 x q3;      odd: q0 x q2
  flex F1 x q3 (4)   even: q2 keys x (copy of qb=q3 queries) -> acc3
                     odd:  q0 keys x (copy of qa=q1 queries) -> acc3

The causal mask (except 128x128 diagonal triangles) enters the score
matmul as 16 extra contraction rows ([K^T; kind] vs [Q^T; qmask]);
diagonal blocks get one static triangular multiplicative mask on the
exp'd scores.  The softmax denominator falls out of the PV matmul via a
ones-column appended to V.  The final divide happens on the host.

x^T streams in as twelve [128, 1024] row-chunk DMAs (2KB lines) in
consumption order; projection matmuls, PSUM->SBUF copies, V transposes
and attention units are ordered so each consumes data as it lands,
overlapping the rest of the stream.  PE warmup matmuls ramp the HAM
clock during the initial DMA latency.
"""

from contextlib import ExitStack

import numpy as np

import concourse.bass as bass
import concourse.mybir as mybir
import concourse.tile as tile
from concourse import bacc
from concourse.masks import make_identity

FP16 = mybir.dt.float16
F32 = mybir.dt.float32

B, T, C, H = 4, 2048, 768, 64
QTR = 512
N_CORES = 8
SCALE = 1.0 / 8.0  # H ** -0.5
MNEG = -30000.0
N_WARM = 10


def build_nc():
    nc = bacc.Bacc("TRN2", target_bir_lowering=False, debug=False,
                   num_devices=N_CORES)
    xts_e = nc.dram_tensor("xts", [C, T], FP16, kind="ExternalInput")
    wkv_e = nc.dram_tensor("wkv", [128, 6 * 128], FP16, kind="ExternalInput")
    wq_e = nc.dram_tensor("wq", [128, 6 * 64], FP16, kind="ExternalInput")
    qsel_e = nc.dram_tensor("qsel", [128, 2], F32, kind="ExternalInput")
    out_e = nc.dram_tensor("out", [65, 3 * QTR], FP16, kind="ExternalOutput")

    with tile.TileContext(nc) as tc, ExitStack() as ctx:
        ep = ctx.enter_context  # shorthand

        const_p = ep(tc.tile_pool(name="const", bufs=1))
        xt_p = ep(tc.tile_pool(name="xt", bufs=1))
        w_p = ep(tc.tile_pool(name="w", bufs=1))
        big_p = ep(tc.tile_pool(name="big", bufs=1))
        acc_ps = ep(tc.tile_pool(name="acc_ps", bufs=1, space="PSUM"))
        exp_p = ep(tc.tile_pool(name="exp", bufs=4))
        o_p = ep(tc.tile_pool(name="o", bufs=2))
        early_ctx = ExitStack()
        pj_ps = early_ctx.enter_context(
            tc.tile_pool(name="pj_ps", bufs=1, space="PSUM"))
        sc_ps = early_ctx.enter_context(
            tc.tile_pool(name="sc_ps", bufs=3, space="PSUM"))

        # ---- DMAs: row-chunk (2KB-line) streams, wave-0 first ----------
        # one tile per chunk so readers wait on exactly their chunk
        xtc = [[xt_p.tile([128, 1024], FP16, name=f"xt{h}{c}")
                for c in range(6)] for h in range(2)]

        def xt_dma(half, c, eng):
            hs = slice(half * 1024, (half + 1) * 1024)
            eng.dma_start(out=xtc[half][c][:, :],
                          in_=xts_e[c * 128:(c + 1) * 128, hs])

        wkv = w_p.tile([128, 6, 128], FP16)
        nc.sync.dma_start(out=wkv[:, :, :],
                          in_=wkv_e[:, :].rearrange("p (n m) -> p n m", m=128))
        wq = w_p.tile([128, 6, 64], FP16)
        nc.scalar.dma_start(out=wq[:, :, :],
                            in_=wq_e[:, :].rearrange("p (n m) -> p n m", m=64))
        for c in range(6):
            xt_dma(0, c, nc.sync if c % 2 == 0 else nc.scalar)
        for c in range(6):
            xt_dma(1, c, nc.sync if c % 2 == 0 else nc.scalar)
        qsel = w_p.tile([128, 2], F32)
        nc.scalar.dma_start(out=qsel[:, :], in_=qsel_e[:, :])
        ktv = big_p.tile([128, T], FP16)      # rows 0:64 K^T, 64:128 V^T
        qt = big_p.tile([64, 3 * QTR], FP16)  # Q^T (a | b | flex)

        # ---- constants (gpsimd; overlaps DMA) + exp table warm ---------
        wtile = const_p.tile([128, QTR], FP16)
        nc.gpsimd.memset(wtile[:, :], 0.0)
        ident64 = const_p.tile([128, 64], FP16)
        make_identity(nc, ident64[64:128, :])
        tri = const_p.tile([128, 128], FP16)
        nc.gpsimd.memset(tri[:, :], 1.0)
        warm = const_p.tile([128, 1], FP16)
        nc.scalar.activation(warm[:, :], tri[:, 0:1],
                             mybir.ActivationFunctionType.Exp, scale=1.0)
        # keep where q - s >= 0  (s = partition/channel, q = free idx)
        nc.gpsimd.affine_select(
            out=tri[:, :], in_=tri[:, :],
            compare_op=mybir.AluOpType.is_ge, fill=0.0,
            base=0, channel_multiplier=-1, pattern=[[1, 128]],
        )

        v = big_p.tile([128, 16 * 65], FP16)
        v3 = v[:, :].rearrange("p (n m) -> p n m", m=65)
        nc.gpsimd.memset(v3[:, :, 64:65], 1.0)

        # ---- PE warmup: start the HAM busy-run while DMAs stream -------
        for i in range(N_WARM):
            wsg = sc_ps.tile([128, QTR], F32, tag="sg", name="wsg")
            nc.tensor.matmul(wsg[:, :], wtile[:, 0:128], wtile[:, :],
                             start=True, stop=True)

        # ---- projections (per 128-row chunk, free dim 512) -------------
        def kv_mm(slot, kv_ps, c):
            half, j = slot // 2, slot % 2
            nc.tensor.matmul(kv_ps[:, :], wkv[:, c, :],
                             xtc[half][c][:, j * QTR:(j + 1) * QTR],
                             start=(c == 0), stop=(c == 5))

        def q_mms(q_ps, c):
            for j in range(2):
                nc.tensor.matmul(
                    q_ps[j * 64:(j + 1) * 64, :], wq[:, c, :],
                    xtc[0][c][:, j * QTR:(j + 1) * QTR],
                    start=(c == 0), stop=(c == 5),
                    skip_group_check=True)

        def kv_copies(slot, kv_ps):
            js = slice(slot * QTR, (slot + 1) * QTR)
            nc.vector.tensor_copy(ktv[:, js], kv_ps[:, :])

        def v_transposes(slot, pool, tag):
            vtp = pool.tile([128, 4 * 64], FP16, tag=tag, name=f"vtp{slot}")
            for i, u in enumerate(range(4 * slot, 4 * slot + 4)):
                nc.tensor.transpose(vtp[:, i * 64:(i + 1) * 64],
                                    ktv[64:128, u * 128:(u + 1) * 128],
                                    ident64[64:128, :])
            nc.vector.tensor_copy(
                v3[:, 4 * slot:4 * slot + 4, 0:64],
                vtp[:, :].rearrange("p (n m) -> p n m", m=64))

        # wave 0 (slots A|B): per chunk c: kv(A), kv(B), Q pair
        kvA = pj_ps.tile([128, QTR], F32, tag="kv0", name="kvA")
        kvB = pj_ps.tile([128, QTR], F32, tag="kv1", name="kvB")
        q_ps = pj_ps.tile([128, QTR], F32, tag="qp", name="q_ps")
        for c in range(6):
            kv_mm(0, kvA, c)
            kv_mm(1, kvB, c)
            q_mms(q_ps, c)
        kv_copies(0, kvA)
        nc.vector.tensor_copy(qt[0:64, 0:QTR], q_ps[0:64, :])
        v_transposes(0, pj_ps, "kv0")
        kv_copies(1, kvB)
        nc.vector.tensor_copy(qt[0:64, QTR:2 * QTR], q_ps[64:128, :])
        # flex query region: qsel-weighted combination of the two halves
        qtmp = o_p.tile([64, QTR], FP16, tag="qtmp", name="qtmp")
        nc.vector.tensor_scalar_mul(qtmp[:, :], qt[0:64, 0:QTR],
                                    qsel[0:64, 0:1])
        nc.vector.tensor_scalar_mul(qt[0:64, 2 * QTR:3 * QTR],
                                    qt[0:64, QTR:2 * QTR], qsel[0:64, 1:2])
        nc.vector.tensor_add(qt[0:64, 2 * QTR:3 * QTR],
                             qt[0:64, 2 * QTR:3 * QTR], qtmp[:, :])
        v_transposes(1, pj_ps, "kv1")

        # ---- attention units -------------------------------------------
        # acc column regions: 0 = quarter a, 1 = quarter b, 2 = flex
        # (flex acc is allocated later, from the post-projection pool)
        accs = [acc_ps.tile([65, QTR], F32, tag=f"acc{q}", name=f"acc{q}")
                for q in range(2)] + [None]
        n_done = [0, 0, 0]
        N_TOTAL = [4, 12, 4]

        def pv(region, u, qoff, free, eg, eoff):
            acc = accs[region]
            nc.tensor.matmul(acc[:, qoff:qoff + free], v3[:, u, :],
                             eg[:, eoff:eoff + free],
                             start=(n_done[region] == 0),
                             stop=(n_done[region] == N_TOTAL[region] - 1))
            n_done[region] += 1

        def attn_unit(region, u, qbase, qoff, free, is_diag):
            sg = sc_ps.tile([128, QTR], F32, tag="sg", name="sg")
            qs = slice(qbase + qoff, qbase + qoff + free)
            nc.tensor.matmul(sg[:, 0:free],
                             ktv[0:64, u * 128:(u + 1) * 128],
                             qt[0:64, qs], start=True, stop=True)
            eg = exp_p.tile([128, QTR], FP16, tag="eg", name="eg")
            nc.scalar.activation(eg[:, 0:free], sg[:, 0:free],
                                 mybir.ActivationFunctionType.Exp,
                                 scale=SCALE)
            if is_diag:
                nc.vector.tensor_mul(eg[:, 0:128], eg[:, 0:128], tri[:, :])
            pv(region, u, qoff, free, eg, 0)

        def tri_unit(quarter, i):
            # unit i of the quarter's diagonal slot; trimmed free dim
            u = quarter * 4 + i
            attn_unit(quarter, u, quarter * QTR, 128 * i, QTR - 128 * i,
                      True)

        def finalize(region):
            acc = accs[region]
            ob = o_p.tile([65, QTR], FP16, tag="ob", name="ob")
            nc.vector.tensor_copy(ob[:, :], acc[:, :])
            nc.sync.dma_start(
                out=out_e[:, region * QTR:(region + 1) * QTR],
                in_=ob[:, :])

        # tri a + b interleaved with wave-1 (F0/F1) projection MMs
        kvF0 = pj_ps.tile([128, QTR], F32, tag="kv0", name="kvF0")
        kvF1 = pj_ps.tile([128, QTR], F32, tag="qp", name="kvF1")
        tri_list = [(0, 0), (0, 1), (0, 2), (0, 3),
                    (1, 0), (1, 1), (1, 2), (1, 3)]
        for g, (q, i) in enumerate(tri_list):
            tri_unit(q, i)
            if g < 6:
                kv_mm(2, kvF0, g)
                kv_mm(3, kvF1, g)
        finalize(0)  # quarter a is tri-only; ship it early

        # full units on slot A (wave-0 data) while wave-1 copies run
        kv_copies(2, kvF0)
        for u in (0, 1):
            attn_unit(1, u, QTR, 0, QTR, False)
        v_transposes(2, pj_ps, "kv1")
        for u in (2, 3):
            attn_unit(1, u, QTR, 0, QTR, False)

        # paired full units (one exp per 2 units) for F0 x qb and flex
        early_ctx.close()
        sc2_ps = ep(tc.tile_pool(name="sc2_ps", bufs=2, space="PSUM"))
        pj2_ps = ep(tc.tile_pool(name="pj2_ps", bufs=1, space="PSUM"))
        accs[2] = pj2_ps.tile([65, QTR], F32, tag="acc2", name="acc2")

        def attn_unit2(region, u, qbase):
            sg = sc2_ps.tile([128, 2 * QTR], F32, tag="sg2", name="sg2")
            nc.tensor.matmul(sg[:, 0:QTR],
                             ktv[0:64, u * 128:(u + 1) * 128],
                             qt[0:64, qbase:qbase + QTR],
                             start=True, stop=True)
            eg = exp_p.tile([128, 2 * QTR], FP16, tag="eg2", name="eg2")
            nc.scalar.activation(eg[:, 0:QTR], sg[:, 0:QTR],
                                 mybir.ActivationFunctionType.Exp,
                                 scale=SCALE)
            pv(region, u, 0, QTR, eg, 0)

        def attn_pair(region, u0, qbase):
            sg = sc2_ps.tile([128, 2 * QTR], F32, tag="sg2", name="sg2")
            for k in range(2):
                nc.tensor.matmul(sg[:, k * QTR:(k + 1) * QTR],
                                 ktv[0:64, (u0 + k) * 128:(u0 + k + 1) * 128],
                                 qt[0:64, qbase:qbase + QTR],
                                 start=True, stop=True)
            eg = exp_p.tile([128, 2 * QTR], FP16, tag="eg2", name="eg2")
            nc.scalar.activation(eg[:, :], sg[:, :],
                                 mybir.ActivationFunctionType.Exp,
                                 scale=SCALE)
            for k in range(2):
                pv(region, u0 + k, 0, QTR, eg, k * QTR)

        kv_copies(3, kvF1)
        attn_pair(1, 8, QTR)
        v_transposes(3, pj2_ps, "vtp3")
        attn_pair(1, 10, QTR)
        finalize(1)
        attn_pair(2, 12, 2 * QTR)
        # last two units single so the tail exp is short
        for u in (14, 15):
            attn_unit2(2, u, 2 * QTR)
        finalize(2)

    nc.compile()
    return nc


# ---------------- host-side shard / unshard ---------------------------

def _tile_weights(w, m):
    """[C, m] -> partition-major [128, 6*m] so the DMA is contiguous."""
    return np.ascontiguousarray(
        w.astype(np.float16).reshape(6, 128, m).transpose(1, 0, 2)
        .reshape(128, 6 * m))


def shard_inputs(x, Wq, Wk, Wv):
    """Full inputs -> list of 8 per-core input dicts."""
    wkv = _tile_weights(np.concatenate([Wk, Wv], axis=1), 128)
    wq16 = _tile_weights(Wq, 64)
    in_maps = []
    for b in range(B):
        xT = np.ascontiguousarray(x[b].astype(np.float16).T)  # [C, T]
        q = [xT[:, i * QTR:(i + 1) * QTR] for i in range(4)]
        for h in range(2):
            if h == 0:
                quarters = [0, 3, 1, 2]       # slots A, B, F0, F1
                flex_q = 3                    # flex queries = quarter b
            else:
                quarters = [1, 2, 0, 0]       # F1 duplicates q0
                flex_q = 1                    # flex queries = quarter a
            xts = np.concatenate([q[i] for i in quarters], axis=1)
            qa, qb = quarters[0], quarters[1]
            qsel = np.zeros((128, 2), np.float32)
            qsel[:, 0] = 1.0 if flex_q == qa else 0.0
            qsel[:, 1] = 1.0 if flex_q == qb else 0.0
            in_maps.append({
                "xts": np.ascontiguousarray(xts),
                "wkv": wkv,
                "wq": wq16,
                "qsel": qsel,
            })
    return in_maps


def unshard_outputs(results):
    """List of 8 per-core result dicts -> full [B, T, H] float32."""
    out = np.zeros((B, T, H), dtype=np.float32)
    for b in range(B):
        for h, (qa, qb) in ((0, (0, 3)), (1, (1, 2))):
            raw = results[2 * b + h]["out"].astype(np.float32)  # [65, 1536]
            acc = [raw[:, 0:QTR], raw[:, QTR:2 * QTR], raw[:, 2 * QTR:]]
            if h == 0:
                acc[1] = acc[1] + acc[2]      # flex -> quarter b
            else:
                acc[0] = acc[0] + acc[2]      # flex -> quarter a
            for col, quarter in ((0, qa), (1, qb)):
                blk = acc[col]
                out[b, quarter * QTR:(quarter + 1) * QTR] = \
                    (blk[0:64, :] / blk[64:65, :]).T
    return out


# ---------------- harness entrypoint ----------------------------------

_NC_CACHE = []


def kernel(x, Wq, Wk, Wv):
    """Full inputs -> full [B, T, H] float32 output, computed on 8 cores."""
    from concourse.bass_utils import run_bass_kernel_spmd

    x = np.asarray(x); Wq = np.asarray(Wq)
    Wk = np.asarray(Wk); Wv = np.asarray(Wv)
    in_maps = shard_inputs(x, Wq, Wk, Wv)
    if not _NC_CACHE:
        _NC_CACHE.append(build_nc())
    nc = _NC_CACHE[0]
    res = run_bass_kernel_spmd(nc, in_maps, core_ids=list(range(N_CORES)))
    return unshard_outputs(res.results)


# revision 16
# speedup vs baseline: 1.1181x; 1.1181x over previous
"""Builder + host-side sharding for the causal attention head kernel (v4).

B=4, T=2048, C=768, H=64 on 8 NeuronCores, pure data parallel (no
collectives).  Each core owns one batch element and two query quarters
(zigzag pairing for causal load balance): core 2b+0 -> quarters (0, 3),
core 2b+1 -> quarters (1, 2).

Per-core device inputs (host-prearranged, fp16):
  xts   [768, 2048]  x[b].T columns laid out as slots [A | B | F0 | F1]
                     (A = qa keys, B = qb keys; diagonal slots)
  wkv   [128, 6*128] [Wk | Wv] pre-tiled partition-major (p, c, m)
  wq    [128, 6*64]  Wq pre-tiled partition-major
  kind  [16, 2048]   key-block indicator rows: kind[j, s] = 1 iff
                     s in [128j, 128j+128)
  qmask [16, 1536]   additive mask values vs the three query regions
                     (qa | qb | flex): -30000 where key-block j is not
                     allowed for that query column
  qsel  [128, 2]     col 0 = 1.0 if the flex query region copies quarter
                     a's Q, col 1 = 1.0 if quarter b's (complementary)

Device output (fp16):
  out   [65, 1536]   cols 0:512 = quarter a acc, 512:1024 = quarter b,
                     1024:1536 = flex acc (host adds it to quarter b on
                     even cores / quarter a on odd cores).  Rows 0:64 =
                     unnormalized attention numerator^T, row 64 =
                     softmax denominator.  Host divides.

Unit structure (20 score/PV units per core, all real work):
  tri A x qa   (4, trimmed free 512/384/256/128, triangular diagonal)
  tri B x qb   (4, trimmed)
  full A x qb  (4)   even: q0 keys x q3; odd: q1 keys x q2
  full F0 x qb (4)   even: q1 x q3;      odd: q0 x q2
  flex F1 x q3 (4)   even: q2 keys x (copy of qb=q3 queries) -> acc3
                     odd:  q0 keys x (copy of qa=q1 queries) -> acc3

The causal mask (except 128x128 diagonal triangles) enters the score
matmul as 16 extra contraction rows ([K^T; kind] vs [Q^T; qmask]);
diagonal blocks get one static triangular multiplicative mask on the
exp'd scores.  The softmax denominator falls out of the PV matmul via a
ones-column appended to V.  The final divide happens on the host.

x^T streams in as twelve [128, 1024] row-chunk DMAs (2KB lines) in
consumption order; projection matmuls, PSUM->SBUF copies, V transposes
and attention units are ordered so each consumes data as it lands,
overlapping the rest of the stream.  PE warmup matmuls ramp the HAM
clock during the initial DMA latency.
"""

from contextlib import ExitStack

import numpy as np

import concourse.bass as bass
import concourse.mybir as mybir
import concourse.tile as tile
from concourse import bacc
from concourse.masks import make_identity

FP16 = mybir.dt.float16
F32 = mybir.dt.float32

B, T, C, H = 4, 2048, 768, 64
QTR = 512
N_CORES = 8
SCALE = 1.0 / 8.0  # H ** -0.5
MNEG = -30000.0
N_WARM = 10


def build_nc():
    nc = bacc.Bacc("TRN2", target_bir_lowering=False, debug=False,
                   num_devices=N_CORES)
    xts_e = nc.dram_tensor("xts", [C, T], FP16, kind="ExternalInput")
    wkv_e = nc.dram_tensor("wkv", [128, 6 * 128], FP16, kind="ExternalInput")
    wq_e = nc.dram_tensor("wq", [128, 6 * 64], FP16, kind="ExternalInput")
    qsel_e = nc.dram_tensor("qsel", [128, 2], F32, kind="ExternalInput")
    out_e = nc.dram_tensor("out", [65, 3 * QTR], FP16, kind="ExternalOutput")

    with tile.TileContext(nc) as tc, ExitStack() as ctx:
        ep = ctx.enter_context  # shorthand

        const_p = ep(tc.tile_pool(name="const", bufs=1))
        xt_p = ep(tc.tile_pool(name="xt", bufs=1))
        w_p = ep(tc.tile_pool(name="w", bufs=1))
        big_p = ep(tc.tile_pool(name="big", bufs=1))
        acc_ps = ep(tc.tile_pool(name="acc_ps", bufs=1, space="PSUM"))
        exp_p = ep(tc.tile_pool(name="exp", bufs=4))
        o_p = ep(tc.tile_pool(name="o", bufs=2))
        early_ctx = ExitStack()
        pj_ps = early_ctx.enter_context(
            tc.tile_pool(name="pj_ps", bufs=1, space="PSUM"))
        sc_ps = early_ctx.enter_context(
            tc.tile_pool(name="sc_ps", bufs=2, space="PSUM"))

        # ---- DMAs: row-chunk (2KB-line) streams, wave-0 first ----------
        # one tile per chunk so readers wait on exactly their chunk
        xtc = [[xt_p.tile([128, 1024], FP16, name=f"xt{h}{c}")
                for c in range(6)] for h in range(2)]

        def xt_dma(half, c, eng):
            hs = slice(half * 1024, (half + 1) * 1024)
            eng.dma_start(out=xtc[half][c][:, :],
                          in_=xts_e[c * 128:(c + 1) * 128, hs])

        wkv = w_p.tile([128, 6, 128], FP16)
        nc.sync.dma_start(out=wkv[:, :, :],
                          in_=wkv_e[:, :].rearrange("p (n m) -> p n m", m=128))
        wq = w_p.tile([128, 6, 64], FP16)
        nc.scalar.dma_start(out=wq[:, :, :],
                            in_=wq_e[:, :].rearrange("p (n m) -> p n m", m=64))
        for c in range(6):
            xt_dma(0, c, nc.sync if c % 2 == 0 else nc.scalar)
        for c in range(6):
            xt_dma(1, c, nc.sync if c % 2 == 0 else nc.scalar)
        qsel = w_p.tile([128, 2], F32)
        nc.scalar.dma_start(out=qsel[:, :], in_=qsel_e[:, :])
        ktv = big_p.tile([128, T], FP16)      # rows 0:64 K^T, 64:128 V^T
        qt = big_p.tile([64, 3 * QTR], FP16)  # Q^T (a | b | flex)

        # ---- constants (gpsimd; overlaps DMA) + exp table warm ---------
        wtile = const_p.tile([128, QTR], FP16)
        nc.gpsimd.memset(wtile[:, :], 0.0)
        ident64 = const_p.tile([128, 64], FP16)
        make_identity(nc, ident64[64:128, :])
        tri = const_p.tile([128, 128], FP16)
        nc.gpsimd.memset(tri[:, :], 1.0)
        warm = const_p.tile([128, 1], FP16)
        nc.scalar.activation(warm[:, :], tri[:, 0:1],
                             mybir.ActivationFunctionType.Exp, scale=1.0)
        # keep where q - s >= 0  (s = partition/channel, q = free idx)
        nc.gpsimd.affine_select(
            out=tri[:, :], in_=tri[:, :],
            compare_op=mybir.AluOpType.is_ge, fill=0.0,
            base=0, channel_multiplier=-1, pattern=[[1, 128]],
        )

        v = big_p.tile([128, 16 * 65], FP16)
        v3 = v[:, :].rearrange("p (n m) -> p n m", m=65)
        nc.gpsimd.memset(v3[:, :, 64:65], 1.0)

        # ---- PE warmup: start the HAM busy-run while DMAs stream -------
        for i in range(N_WARM):
            wsg = sc_ps.tile([128, QTR], F32, tag="sg", name="wsg")
            nc.tensor.matmul(wsg[:, :], wtile[:, 0:128], wtile[:, :],
                             start=True, stop=True)

        # ---- projections (per 128-row chunk, free dim 512) -------------
        def kv_mm(slot, kv_ps, c):
            half, j = slot // 2, slot % 2
            nc.tensor.matmul(kv_ps[:, :], wkv[:, c, :],
                             xtc[half][c][:, j * QTR:(j + 1) * QTR],
                             start=(c == 0), stop=(c == 5))

        def q_mms(q_ps, c):
            for j in range(2):
                nc.tensor.matmul(
                    q_ps[j * 64:(j + 1) * 64, :], wq[:, c, :],
                    xtc[0][c][:, j * QTR:(j + 1) * QTR],
                    start=(c == 0), stop=(c == 5),
                    skip_group_check=True)

        def kv_copies(slot, kv_ps):
            js = slice(slot * QTR, (slot + 1) * QTR)
            nc.vector.tensor_copy(ktv[:, js], kv_ps[:, :])

        def v_transposes(slot, pool, tag):
            vtp = pool.tile([128, 4 * 64], FP16, tag=tag, name=f"vtp{slot}")
            for i, u in enumerate(range(4 * slot, 4 * slot + 4)):
                nc.tensor.transpose(vtp[:, i * 64:(i + 1) * 64],
                                    ktv[64:128, u * 128:(u + 1) * 128],
                                    ident64[64:128, :])
            nc.vector.tensor_copy(
                v3[:, 4 * slot:4 * slot + 4, 0:64],
                vtp[:, :].rearrange("p (n m) -> p n m", m=64))

        # wave 0 (slots A|B): per chunk c: kv(A), kv(B), Q pair
        kvA = pj_ps.tile([128, QTR], F32, tag="kv0", name="kvA")
        kvB = pj_ps.tile([128, QTR], F32, tag="kv1", name="kvB")
        q_ps = pj_ps.tile([128, QTR], F32, tag="qp", name="q_ps")
        for c in range(6):
            kv_mm(0, kvA, c)
            kv_mm(1, kvB, c)
            q_mms(q_ps, c)
        kv_copies(0, kvA)
        nc.vector.tensor_copy(qt[0:64, 0:QTR], q_ps[0:64, :])
        v_transposes(0, pj_ps, "vtp")
        kv_copies(1, kvB)
        nc.vector.tensor_copy(qt[0:64, QTR:2 * QTR], q_ps[64:128, :])
        # flex query region: qsel-weighted combination of the two halves
        qtmp = o_p.tile([64, QTR], FP16, tag="qtmp", name="qtmp")
        nc.vector.tensor_scalar_mul(qtmp[:, :], qt[0:64, 0:QTR],
                                    qsel[0:64, 0:1])
        nc.vector.tensor_scalar_mul(qt[0:64, 2 * QTR:3 * QTR],
                                    qt[0:64, QTR:2 * QTR], qsel[0:64, 1:2])
        nc.vector.tensor_add(qt[0:64, 2 * QTR:3 * QTR],
                             qt[0:64, 2 * QTR:3 * QTR], qtmp[:, :])
        v_transposes(1, pj_ps, "vtp")

        # ---- attention units -------------------------------------------
        # acc column regions: 0 = quarter a, 1 = quarter b, 2 = flex
        # (flex acc is allocated later, from the post-projection pool)
        accs = [acc_ps.tile([65, QTR], F32, tag=f"acc{q}", name=f"acc{q}")
                for q in range(2)] + [None]
        n_done = [0, 0, 0]
        N_TOTAL = [4, 12, 4]

        def pv(region, u, qoff, free, eg, eoff):
            acc = accs[region]
            nc.tensor.matmul(acc[:, qoff:qoff + free], v3[:, u, :],
                             eg[:, eoff:eoff + free],
                             start=(n_done[region] == 0),
                             stop=(n_done[region] == N_TOTAL[region] - 1))
            n_done[region] += 1

        def attn_unit(region, u, qbase, qoff, free, is_diag):
            sg = sc_ps.tile([128, QTR], F32, tag="sg", name="sg")
            qs = slice(qbase + qoff, qbase + qoff + free)
            nc.tensor.matmul(sg[:, 0:free],
                             ktv[0:64, u * 128:(u + 1) * 128],
                             qt[0:64, qs], start=True, stop=True)
            eg = exp_p.tile([128, QTR], FP16, tag="eg", name="eg")
            nc.scalar.activation(eg[:, 0:free], sg[:, 0:free],
                                 mybir.ActivationFunctionType.Exp,
                                 scale=SCALE)
            if is_diag:
                nc.vector.tensor_mul(eg[:, 0:128], eg[:, 0:128], tri[:, :])
            pv(region, u, qoff, free, eg, 0)

        def tri_unit(quarter, i):
            # unit i of the quarter's diagonal slot; trimmed free dim
            u = quarter * 4 + i
            attn_unit(quarter, u, quarter * QTR, 128 * i, QTR - 128 * i,
                      True)

        def finalize(region):
            acc = accs[region]
            ob = o_p.tile([65, QTR], FP16, tag="ob", name="ob")
            nc.vector.tensor_copy(ob[:, :], acc[:, :])
            nc.sync.dma_start(
                out=out_e[:, region * QTR:(region + 1) * QTR],
                in_=ob[:, :])

        # tri a + b interleaved with wave-1 (F0/F1) projection MMs
        kvF0 = pj_ps.tile([128, QTR], F32, tag="kv0", name="kvF0")
        kvF1 = pj_ps.tile([128, QTR], F32, tag="qp", name="kvF1")
        tri_list = [(0, 0), (0, 1), (0, 2), (0, 3),
                    (1, 0), (1, 1), (1, 2), (1, 3)]
        for g, (q, i) in enumerate(tri_list):
            tri_unit(q, i)
            if g < 6:
                kv_mm(2, kvF0, g)
                kv_mm(3, kvF1, g)
        finalize(0)  # quarter a is tri-only; ship it early

        # full units on slot A (wave-0 data) while wave-1 copies run
        kv_copies(2, kvF0)
        for u in (0, 1):
            attn_unit(1, u, QTR, 0, QTR, False)
        v_transposes(2, pj_ps, "vtp")
        for u in (2, 3):
            attn_unit(1, u, QTR, 0, QTR, False)

        # paired full units (one exp per 2 units) for F0 x qb and flex
        early_ctx.close()
        sc2_ps = ep(tc.tile_pool(name="sc2_ps", bufs=2, space="PSUM"))
        pj2_ps = ep(tc.tile_pool(name="pj2_ps", bufs=1, space="PSUM"))
        accs[2] = pj2_ps.tile([65, QTR], F32, tag="acc2", name="acc2")

        def attn_unit2(region, u, qbase):
            sg = sc2_ps.tile([128, 2 * QTR], F32, tag="sg2", name="sg2")
            nc.tensor.matmul(sg[:, 0:QTR],
                             ktv[0:64, u * 128:(u + 1) * 128],
                             qt[0:64, qbase:qbase + QTR],
                             start=True, stop=True)
            eg = exp_p.tile([128, 2 * QTR], FP16, tag="eg2", name="eg2")
            nc.scalar.activation(eg[:, 0:QTR], sg[:, 0:QTR],
                                 mybir.ActivationFunctionType.Exp,
                                 scale=SCALE)
            pv(region, u, 0, QTR, eg, 0)

        def attn_pair(region, u0, qbase):
            sg = sc2_ps.tile([128, 2 * QTR], F32, tag="sg2", name="sg2")
            for k in range(2):
                nc.tensor.matmul(sg[:, k * QTR:(k + 1) * QTR],
                                 ktv[0:64, (u0 + k) * 128:(u0 + k + 1) * 128],
                                 qt[0:64, qbase:qbase + QTR],
                                 start=True, stop=True)
            eg = exp_p.tile([128, 2 * QTR], FP16, tag="eg2", name="eg2")
            nc.scalar.activation(eg[:, :], sg[:, :],
                                 mybir.ActivationFunctionType.Exp,
                                 scale=SCALE)
            for k in range(2):
                pv(region, u0 + k, 0, QTR, eg, k * QTR)

        kv_copies(3, kvF1)
        attn_pair(1, 8, QTR)
        v_transposes(3, pj2_ps, "vtp3")
        attn_pair(1, 10, QTR)
        finalize(1)
        attn_pair(2, 12, 2 * QTR)
        # last two units single so the tail exp is short
        for u in (14, 15):
            attn_unit2(2, u, 2 * QTR)
        finalize(2)

    nc.compile()
    return nc


# ---------------- host-side shard / unshard ---------------------------

def _tile_weights(w, m):
    """[C, m] -> partition-major [128, 6*m] so the DMA is contiguous."""
    return np.ascontiguousarray(
        w.astype(np.float16).reshape(6, 128, m).transpose(1, 0, 2)
        .reshape(128, 6 * m))


def shard_inputs(x, Wq, Wk, Wv):
    """Full inputs -> list of 8 per-core input dicts."""
    wkv = _tile_weights(np.concatenate([Wk, Wv], axis=1), 128)
    wq16 = _tile_weights(Wq, 64)
    in_maps = []
    for b in range(B):
        xT = np.ascontiguousarray(x[b].astype(np.float16).T)  # [C, T]
        q = [xT[:, i * QTR:(i + 1) * QTR] for i in range(4)]
        for h in range(2):
            if h == 0:
                quarters = [0, 3, 1, 2]       # slots A, B, F0, F1
                flex_q = 3                    # flex queries = quarter b
            else:
                quarters = [1, 2, 0, 0]       # F1 duplicates q0
                flex_q = 1                    # flex queries = quarter a
            xts = np.concatenate([q[i] for i in quarters], axis=1)
            qa, qb = quarters[0], quarters[1]
            qsel = np.zeros((128, 2), np.float32)
            qsel[:, 0] = 1.0 if flex_q == qa else 0.0
            qsel[:, 1] = 1.0 if flex_q == qb else 0.0
            in_maps.append({
                "xts": np.ascontiguousarray(xts),
                "wkv": wkv,
                "wq": wq16,
                "qsel": qsel,
            })
    return in_maps


def unshard_outputs(results):
    """List of 8 per-core result dicts -> full [B, T, H] float32."""
    out = np.zeros((B, T, H), dtype=np.float32)
    for b in range(B):
        for h, (qa, qb) in ((0, (0, 3)), (1, (1, 2))):
            raw = results[2 * b + h]["out"].astype(np.float32)  # [65, 1536]
            acc = [raw[:, 0:QTR], raw[:, QTR:2 * QTR], raw[:, 2 * QTR:]]
            if h == 0:
                acc[1] = acc[1] + acc[2]      # flex -> quarter b
            else:
                acc[0] = acc[0] + acc[2]      # flex -> quarter a
            for col, quarter in ((0, qa), (1, qb)):
                blk = acc[col]
                out[b, quarter * QTR:(quarter + 1) * QTR] = \
                    (blk[0:64, :] / blk[64:65, :]).T
    return out


# ---------------- harness entrypoint ----------------------------------

_NC_CACHE = []


def kernel(x, Wq, Wk, Wv):
    """Full inputs -> full [B, T, H] float32 output, computed on 8 cores."""
    from concourse.bass_utils import run_bass_kernel_spmd

    x = np.asarray(x); Wq = np.asarray(Wq)
    Wk = np.asarray(Wk); Wv = np.asarray(Wv)
    in_maps = shard_inputs(x, Wq, Wk, Wv)
    if not _NC_CACHE:
        _NC_CACHE.append(build_nc())
    nc = _NC_CACHE[0]
    res = run_bass_kernel_spmd(nc, in_maps, core_ids=list(range(N_CORES)))
    return unshard_outputs(res.results)


# revision 17
# speedup vs baseline: 1.1203x; 1.0020x over previous
"""Builder + host-side sharding for the causal attention head kernel (v4).

B=4, T=2048, C=768, H=64 on 8 NeuronCores, pure data parallel (no
collectives).  Each core owns one batch element and two query quarters
(zigzag pairing for causal load balance): core 2b+0 -> quarters (0, 3),
core 2b+1 -> quarters (1, 2).

Per-core device inputs (host-prearranged, fp16):
  xts   [768, 2048]  x[b].T columns laid out as slots [A | B | F0 | F1]
                     (A = qa keys, B = qb keys; diagonal slots)
  wkv   [128, 6*128] [Wk | Wv] pre-tiled partition-major (p, c, m)
  wq    [128, 6*64]  Wq pre-tiled partition-major
  kind  [16, 2048]   key-block indicator rows: kind[j, s] = 1 iff
                     s in [128j, 128j+128)
  qmask [16, 1536]   additive mask values vs the three query regions
                     (qa | qb | flex): -30000 where key-block j is not
                     allowed for that query column
  qsel  [128, 2]     col 0 = 1.0 if the flex query region copies quarter
                     a's Q, col 1 = 1.0 if quarter b's (complementary)

Device output (fp16):
  out   [65, 1536]   cols 0:512 = quarter a acc, 512:1024 = quarter b,
                     1024:1536 = flex acc (host adds it to quarter b on
                     even cores / quarter a on odd cores).  Rows 0:64 =
                     unnormalized attention numerator^T, row 64 =
                     softmax denominator.  Host divides.

Unit structure (20 score/PV units per core, all real work):
  tri A x qa   (4, trimmed free 512/384/256/128, triangular diagonal)
  tri B x qb   (4, trimmed)
  full A x qb  (4)   even: q0 keys x q3; odd: q1 keys x q2
  full F0 x qb (4)   even: q1 x q3;      odd: q0 x q2
  flex F1 x q3 (4)   even: q2 keys x (copy of qb=q3 queries) -> acc3
                     odd:  q0 keys x (copy of qa=q1 queries) -> acc3

The causal mask (except 128x128 diagonal triangles) enters the score
matmul as 16 extra contraction rows ([K^T; kind] vs [Q^T; qmask]);
diagonal blocks get one static triangular multiplicative mask on the
exp'd scores.  The softmax denominator falls out of the PV matmul via a
ones-column appended to V.  The final divide happens on the host.

x^T streams in as twelve [128, 1024] row-chunk DMAs (2KB lines) in
consumption order; projection matmuls, PSUM->SBUF copies, V transposes
and attention units are ordered so each consumes data as it lands,
overlapping the rest of the stream.  PE warmup matmuls ramp the HAM
clock during the initial DMA latency.
"""

from contextlib import ExitStack

import numpy as np

import concourse.bass as bass
import concourse.mybir as mybir
import concourse.tile as tile
from concourse import bacc
from concourse.masks import make_identity

FP16 = mybir.dt.float16
F32 = mybir.dt.float32

B, T, C, H = 4, 2048, 768, 64
QTR = 512
N_CORES = 8
SCALE = 1.0 / 8.0  # H ** -0.5
MNEG = -30000.0
N_WARM = 13


def build_nc():
    nc = bacc.Bacc("TRN2", target_bir_lowering=False, debug=False,
                   num_devices=N_CORES)
    xts_e = nc.dram_tensor("xts", [C, T], FP16, kind="ExternalInput")
    wkv_e = nc.dram_tensor("wkv", [128, 6 * 128], FP16, kind="ExternalInput")
    wq_e = nc.dram_tensor("wq", [128, 6 * 64], FP16, kind="ExternalInput")
    qsel_e = nc.dram_tensor("qsel", [128, 2], F32, kind="ExternalInput")
    out_e = nc.dram_tensor("out", [65, 3 * QTR], FP16, kind="ExternalOutput")

    with tile.TileContext(nc) as tc, ExitStack() as ctx:
        ep = ctx.enter_context  # shorthand

        const_p = ep(tc.tile_pool(name="const", bufs=1))
        xt_p = ep(tc.tile_pool(name="xt", bufs=1))
        w_p = ep(tc.tile_pool(name="w", bufs=1))
        big_p = ep(tc.tile_pool(name="big", bufs=1))
        acc_ps = ep(tc.tile_pool(name="acc_ps", bufs=1, space="PSUM"))
        exp_p = ep(tc.tile_pool(name="exp", bufs=4))
        o_p = ep(tc.tile_pool(name="o", bufs=2))
        early_ctx = ExitStack()
        pj_ps = early_ctx.enter_context(
            tc.tile_pool(name="pj_ps", bufs=1, space="PSUM"))
        sc_ps = early_ctx.enter_context(
            tc.tile_pool(name="sc_ps", bufs=2, space="PSUM"))

        # ---- DMAs: row-chunk (2KB-line) streams, wave-0 first ----------
        # one tile per chunk so readers wait on exactly their chunk
        xtc = [[xt_p.tile([128, 1024], FP16, name=f"xt{h}{c}")
                for c in range(6)] for h in range(2)]

        def xt_dma(half, c, eng):
            hs = slice(half * 1024, (half + 1) * 1024)
            eng.dma_start(out=xtc[half][c][:, :],
                          in_=xts_e[c * 128:(c + 1) * 128, hs])

        wkv = w_p.tile([128, 6, 128], FP16)
        nc.sync.dma_start(out=wkv[:, :, :],
                          in_=wkv_e[:, :].rearrange("p (n m) -> p n m", m=128))
        wq = w_p.tile([128, 6, 64], FP16)
        nc.scalar.dma_start(out=wq[:, :, :],
                            in_=wq_e[:, :].rearrange("p (n m) -> p n m", m=64))
        for c in range(6):
            xt_dma(0, c, nc.sync if c % 2 == 0 else nc.scalar)
        for c in range(6):
            xt_dma(1, c, nc.sync if c % 2 == 0 else nc.scalar)
        qsel = w_p.tile([128, 2], F32)
        nc.scalar.dma_start(out=qsel[:, :], in_=qsel_e[:, :])
        ktv = big_p.tile([128, T], FP16)      # rows 0:64 K^T, 64:128 V^T
        qt = big_p.tile([64, 3 * QTR], FP16)  # Q^T (a | b | flex)

        # ---- constants (gpsimd; overlaps DMA) + exp table warm ---------
        wtile = const_p.tile([128, QTR], FP16)
        nc.gpsimd.memset(wtile[:, :], 0.0)
        ident64 = const_p.tile([128, 64], FP16)
        make_identity(nc, ident64[64:128, :])
        tri = const_p.tile([128, 128], FP16)
        nc.gpsimd.memset(tri[:, :], 1.0)
        warm = const_p.tile([128, 1], FP16)
        nc.scalar.activation(warm[:, :], tri[:, 0:1],
                             mybir.ActivationFunctionType.Exp, scale=1.0)
        # keep where q - s >= 0  (s = partition/channel, q = free idx)
        nc.gpsimd.affine_select(
            out=tri[:, :], in_=tri[:, :],
            compare_op=mybir.AluOpType.is_ge, fill=0.0,
            base=0, channel_multiplier=-1, pattern=[[1, 128]],
        )

        v = big_p.tile([128, 16 * 65], FP16)
        v3 = v[:, :].rearrange("p (n m) -> p n m", m=65)
        nc.gpsimd.memset(v3[:, :, 64:65], 1.0)

        # ---- PE warmup: start the HAM busy-run while DMAs stream -------
        for i in range(N_WARM):
            wsg = sc_ps.tile([128, QTR], F32, tag="sg", name="wsg")
            nc.tensor.matmul(wsg[:, :], wtile[:, 0:128], wtile[:, :],
                             start=True, stop=True)

        # ---- projections (per 128-row chunk, free dim 512) -------------
        def kv_mm(slot, kv_ps, c):
            half, j = slot // 2, slot % 2
            nc.tensor.matmul(kv_ps[:, :], wkv[:, c, :],
                             xtc[half][c][:, j * QTR:(j + 1) * QTR],
                             start=(c == 0), stop=(c == 5))

        def q_mms(q_ps, c):
            for j in range(2):
                nc.tensor.matmul(
                    q_ps[j * 64:(j + 1) * 64, :], wq[:, c, :],
                    xtc[0][c][:, j * QTR:(j + 1) * QTR],
                    start=(c == 0), stop=(c == 5),
                    skip_group_check=True)

        def kv_copies(slot, kv_ps):
            js = slice(slot * QTR, (slot + 1) * QTR)
            nc.vector.tensor_copy(ktv[:, js], kv_ps[:, :])

        def v_transposes(slot, pool, tag):
            vtp = pool.tile([128, 4 * 64], FP16, tag=tag, name=f"vtp{slot}")
            for i, u in enumerate(range(4 * slot, 4 * slot + 4)):
                nc.tensor.transpose(vtp[:, i * 64:(i + 1) * 64],
                                    ktv[64:128, u * 128:(u + 1) * 128],
                                    ident64[64:128, :])
            nc.vector.tensor_copy(
                v3[:, 4 * slot:4 * slot + 4, 0:64],
                vtp[:, :].rearrange("p (n m) -> p n m", m=64))

        # wave 0 (slots A|B): per chunk c: kv(A), kv(B), Q pair
        kvA = pj_ps.tile([128, QTR], F32, tag="kv0", name="kvA")
        kvB = pj_ps.tile([128, QTR], F32, tag="kv1", name="kvB")
        q_ps = pj_ps.tile([128, QTR], F32, tag="qp", name="q_ps")
        for c in range(6):
            kv_mm(0, kvA, c)
            kv_mm(1, kvB, c)
            q_mms(q_ps, c)
        kv_copies(0, kvA)
        nc.vector.tensor_copy(qt[0:64, 0:QTR], q_ps[0:64, :])
        v_transposes(0, pj_ps, "vtp")
        kv_copies(1, kvB)
        nc.vector.tensor_copy(qt[0:64, QTR:2 * QTR], q_ps[64:128, :])
        # flex query region: qsel-weighted combination of the two halves
        qtmp = o_p.tile([64, QTR], FP16, tag="qtmp", name="qtmp")
        nc.vector.tensor_scalar_mul(qtmp[:, :], qt[0:64, 0:QTR],
                                    qsel[0:64, 0:1])
        nc.vector.tensor_scalar_mul(qt[0:64, 2 * QTR:3 * QTR],
                                    qt[0:64, QTR:2 * QTR], qsel[0:64, 1:2])
        nc.vector.tensor_add(qt[0:64, 2 * QTR:3 * QTR],
                             qt[0:64, 2 * QTR:3 * QTR], qtmp[:, :])
        v_transposes(1, pj_ps, "vtp")

        # ---- attention units -------------------------------------------
        # acc column regions: 0 = quarter a, 1 = quarter b, 2 = flex
        # (flex acc is allocated later, from the post-projection pool)
        accs = [acc_ps.tile([65, QTR], F32, tag=f"acc{q}", name=f"acc{q}")
                for q in range(2)] + [None]
        n_done = [0, 0, 0]
        N_TOTAL = [4, 12, 4]

        def pv(region, u, qoff, free, eg, eoff):
            acc = accs[region]
            nc.tensor.matmul(acc[:, qoff:qoff + free], v3[:, u, :],
                             eg[:, eoff:eoff + free],
                             start=(n_done[region] == 0),
                             stop=(n_done[region] == N_TOTAL[region] - 1))
            n_done[region] += 1

        def attn_unit(region, u, qbase, qoff, free, is_diag):
            sg = sc_ps.tile([128, QTR], F32, tag="sg", name="sg")
            qs = slice(qbase + qoff, qbase + qoff + free)
            nc.tensor.matmul(sg[:, 0:free],
                             ktv[0:64, u * 128:(u + 1) * 128],
                             qt[0:64, qs], start=True, stop=True)
            eg = exp_p.tile([128, QTR], FP16, tag="eg", name="eg")
            nc.scalar.activation(eg[:, 0:free], sg[:, 0:free],
                                 mybir.ActivationFunctionType.Exp,
                                 scale=SCALE)
            if is_diag:
                nc.gpsimd.tensor_mul(eg[:, 0:128], eg[:, 0:128], tri[:, :])
            pv(region, u, qoff, free, eg, 0)

        def tri_unit(quarter, i):
            # unit i of the quarter's diagonal slot; trimmed free dim
            u = quarter * 4 + i
            attn_unit(quarter, u, quarter * QTR, 128 * i, QTR - 128 * i,
                      True)

        def finalize(region):
            acc = accs[region]
            ob = o_p.tile([65, QTR], FP16, tag="ob", name="ob")
            nc.vector.tensor_copy(ob[:, :], acc[:, :])
            nc.sync.dma_start(
                out=out_e[:, region * QTR:(region + 1) * QTR],
                in_=ob[:, :])

        # tri a + b interleaved with wave-1 (F0/F1) projection MMs
        kvF0 = pj_ps.tile([128, QTR], F32, tag="kv0", name="kvF0")
        kvF1 = pj_ps.tile([128, QTR], F32, tag="qp", name="kvF1")
        tri_list = [(0, 0), (0, 1), (0, 2), (0, 3),
                    (1, 0), (1, 1), (1, 2), (1, 3)]
        for g, (q, i) in enumerate(tri_list):
            tri_unit(q, i)
            if g < 6:
                kv_mm(2, kvF0, g)
                kv_mm(3, kvF1, g)
        finalize(0)  # quarter a is tri-only; ship it early

        # full units on slot A (wave-0 data) while wave-1 copies run
        kv_copies(2, kvF0)
        for u in (0, 1):
            attn_unit(1, u, QTR, 0, QTR, False)
        v_transposes(2, pj_ps, "vtp")
        for u in (2, 3):
            attn_unit(1, u, QTR, 0, QTR, False)

        # paired full units (one exp per 2 units) for F0 x qb and flex
        early_ctx.close()
        sc2_ps = ep(tc.tile_pool(name="sc2_ps", bufs=2, space="PSUM"))
        pj2_ps = ep(tc.tile_pool(name="pj2_ps", bufs=1, space="PSUM"))
        accs[2] = pj2_ps.tile([65, QTR], F32, tag="acc2", name="acc2")

        def attn_unit2(region, u, qbase):
            sg = sc2_ps.tile([128, 2 * QTR], F32, tag="sg2", name="sg2")
            nc.tensor.matmul(sg[:, 0:QTR],
                             ktv[0:64, u * 128:(u + 1) * 128],
                             qt[0:64, qbase:qbase + QTR],
                             start=True, stop=True)
            eg = exp_p.tile([128, 2 * QTR], FP16, tag="eg2", name="eg2")
            nc.scalar.activation(eg[:, 0:QTR], sg[:, 0:QTR],
                                 mybir.ActivationFunctionType.Exp,
                                 scale=SCALE)
            pv(region, u, 0, QTR, eg, 0)

        def attn_pair(region, u0, qbase):
            sg = sc2_ps.tile([128, 2 * QTR], F32, tag="sg2", name="sg2")
            for k in range(2):
                nc.tensor.matmul(sg[:, k * QTR:(k + 1) * QTR],
                                 ktv[0:64, (u0 + k) * 128:(u0 + k + 1) * 128],
                                 qt[0:64, qbase:qbase + QTR],
                                 start=True, stop=True)
            eg = exp_p.tile([128, 2 * QTR], FP16, tag="eg2", name="eg2")
            nc.scalar.activation(eg[:, :], sg[:, :],
                                 mybir.ActivationFunctionType.Exp,
                                 scale=SCALE)
            for k in range(2):
                pv(region, u0 + k, 0, QTR, eg, k * QTR)

        kv_copies(3, kvF1)
        attn_pair(1, 8, QTR)
        v_transposes(3, pj2_ps, "vtp3")
        attn_pair(1, 10, QTR)
        finalize(1)
        attn_pair(2, 12, 2 * QTR)
        # last two units single so the tail exp is short
        for u in (14, 15):
            attn_unit2(2, u, 2 * QTR)
        finalize(2)

    nc.compile()
    return nc


# ---------------- host-side shard / unshard ---------------------------

def _tile_weights(w, m):
    """[C, m] -> partition-major [128, 6*m] so the DMA is contiguous."""
    return np.ascontiguousarray(
        w.astype(np.float16).reshape(6, 128, m).transpose(1, 0, 2)
        .reshape(128, 6 * m))


def shard_inputs(x, Wq, Wk, Wv):
    """Full inputs -> list of 8 per-core input dicts."""
    wkv = _tile_weights(np.concatenate([Wk, Wv], axis=1), 128)
    wq16 = _tile_weights(Wq, 64)
    in_maps = []
    for b in range(B):
        xT = np.ascontiguousarray(x[b].astype(np.float16).T)  # [C, T]
        q = [xT[:, i * QTR:(i + 1) * QTR] for i in range(4)]
        for h in range(2):
            if h == 0:
                quarters = [0, 3, 1, 2]       # slots A, B, F0, F1
                flex_q = 3                    # flex queries = quarter b
            else:
                quarters = [1, 2, 0, 0]       # F1 duplicates q0
                flex_q = 1                    # flex queries = quarter a
            xts = np.concatenate([q[i] for i in quarters], axis=1)
            qa, qb = quarters[0], quarters[1]
            qsel = np.zeros((128, 2), np.float32)
            qsel[:, 0] = 1.0 if flex_q == qa else 0.0
            qsel[:, 1] = 1.0 if flex_q == qb else 0.0
            in_maps.append({
                "xts": np.ascontiguousarray(xts),
                "wkv": wkv,
                "wq": wq16,
                "qsel": qsel,
            })
    return in_maps


def unshard_outputs(results):
    """List of 8 per-core result dicts -> full [B, T, H] float32."""
    out = np.zeros((B, T, H), dtype=np.float32)
    for b in range(B):
        for h, (qa, qb) in ((0, (0, 3)), (1, (1, 2))):
            raw = results[2 * b + h]["out"].astype(np.float32)  # [65, 1536]
            acc = [raw[:, 0:QTR], raw[:, QTR:2 * QTR], raw[:, 2 * QTR:]]
            if h == 0:
                acc[1] = acc[1] + acc[2]      # flex -> quarter b
            else:
                acc[0] = acc[0] + acc[2]      # flex -> quarter a
            for col, quarter in ((0, qa), (1, qb)):
                blk = acc[col]
                out[b, quarter * QTR:(quarter + 1) * QTR] = \
                    (blk[0:64, :] / blk[64:65, :]).T
    return out


# ---------------- harness entrypoint ----------------------------------

_NC_CACHE = []


def kernel(x, Wq, Wk, Wv):
    """Full inputs -> full [B, T, H] float32 output, computed on 8 cores."""
    from concourse.bass_utils import run_bass_kernel_spmd

    x = np.asarray(x); Wq = np.asarray(Wq)
    Wk = np.asarray(Wk); Wv = np.asarray(Wv)
    in_maps = shard_inputs(x, Wq, Wk, Wv)
    if not _NC_CACHE:
        _NC_CACHE.append(build_nc())
    nc = _NC_CACHE[0]
    res = run_bass_kernel_spmd(nc, in_maps, core_ids=list(range(N_CORES)))
    return unshard_outputs(res.results)
